# revision 1
# baseline (speedup 1.0000x reference)
"""Disentangled attention (fused common+personal QKV projections + MHA) on 8 TRN2 cores.

Strategy: data-parallel over batch N=8 (one batch element per NeuronCore, zero
communication). Host pre-sums W_c+W_p / b_c+b_p (exact), casts x/W to bf16, and
pre-transposes x so the device only sees x^T.

Per-core device pipeline (S=1024, D=512, H=8, hd=64):
  phase 1: qT = (W_q)^T-style projections producing q^T,k^T [D,S] and v [S,D]
           (bf16 matmuls, fp32 PSUM accumulate, bias added on PSUM evacuation)
  phase 2: per head: energy^T[sk,sq] = (kT tile)^T-free matmuls; exp on ScalarE
           (scale 1/sqrt(D) folded into activation; softmax max-subtraction
           skipped -- |energy/sqrt(D)| <~ 4.5 for these inputs);
           attn@V with stationary [v_h | ones | 0-pad] (80 cols) giving
           out'^T[80,sq] with row 64 = softmax denominator; DMA-xbar transpose
           back to [sq,80]; VectorE reciprocal + broadcast multiply normalizes.
"""

import os
from contextlib import ExitStack

import numpy as np
import ml_dtypes

import concourse.bass as bass
import concourse.tile as tile
import concourse.mybir as mybir
from concourse import bacc
from concourse.bass_utils import run_bass_kernel_spmd

B, S, D, H, HD = 8, 1024, 512, 8, 64
P = 128
KB = D // P           # 4 contraction blocks
SB = S // P           # 8 sequence tiles
VW = 80               # v-tile width: 64 data + 1 ones + 15 pad (xbar needs %16)
BF16 = mybir.dt.bfloat16
F32 = mybir.dt.float32
SCALE = 1.0 / float(np.sqrt(D))

NPBF16 = ml_dtypes.bfloat16


def _bcast_ap(ap, parts):
    """Broadcast a [1, ...] AP across `parts` partitions (stride-0 partition dim)."""
    return bass.AP(tensor=ap.tensor, offset=ap.offset, ap=[[0, parts]] + list(ap.ap[1:]))


def emit_kernel(ctx: ExitStack, tc: tile.TileContext):
    nc = tc.nc

    # inputs arrive host-preblocked as [p, k, :] so each loads as ONE DMA with
    # 4-8KB contiguous runs on both sides (load phase is packet-latency bound)
    xT_d = nc.dram_tensor("xT", [P, KB, S], BF16, kind="ExternalInput")
    wq_d = nc.dram_tensor("wq", [P, KB, D], BF16, kind="ExternalInput")
    wk_d = nc.dram_tensor("wk", [P, KB, D], BF16, kind="ExternalInput")
    wv_d = nc.dram_tensor("wv", [P, KB, D], BF16, kind="ExternalInput")
    bq_d = nc.dram_tensor("bq", [P, KB], F32, kind="ExternalInput")
    bk_d = nc.dram_tensor("bk", [P, KB], F32, kind="ExternalInput")
    bv_d = nc.dram_tensor("bv", [1, D], F32, kind="ExternalInput")
    out_d = nc.dram_tensor("out", [S, D], BF16, kind="ExternalOutput")

    consts = ctx.enter_context(tc.tile_pool(name="consts", bufs=1))
    persist = ctx.enter_context(tc.tile_pool(name="persist", bufs=1))

    # ---- load inputs ----
    xT_sb = persist.tile([P, KB, S], BF16, tag="xT", name="xT")
    wq_sb = persist.tile([P, KB, D], BF16, tag="wq", name="wq")
    wk_sb = persist.tile([P, KB, D], BF16, tag="wk", name="wk")
    wv_sb = persist.tile([P, KB, D], BF16, tag="wv", name="wv")
    # ordering: xT + wq + wk first (q/k projections gate everything); xT on the
    # sync HWDGE queue, weights on the scalar queue, so they load in parallel
    bq_sb = consts.tile([P, KB], F32, tag="bq", name="bq")
    bk_sb = consts.tile([P, KB], F32, tag="bk", name="bk")
    bv_sb = consts.tile([P, D], F32, tag="bv", name="bv")
    # queue-balanced load split (~0.67MB critical bytes per queue): the wire is
    # per-queue throughput-capped, so spread xT halves across all three paths
    nc.sync.dma_start(out=xT_sb[:, 0:2, :], in_=xT_d[:, 0:2, :])
    nc.scalar.dma_start(out=wq_sb[:], in_=wq_d[:])
    nc.gpsimd.dma_start(out=wk_sb[:], in_=wk_d[:])
    nc.scalar.dma_start(out=xT_sb[:, 2:3, :], in_=xT_d[:, 2:3, :])
    nc.gpsimd.dma_start(out=xT_sb[:, 3:4, :], in_=xT_d[:, 3:4, :])
    nc.sync.dma_start(out=bq_sb[:], in_=bq_d[:])
    nc.sync.dma_start(out=bk_sb[:], in_=bk_d[:])
    nc.scalar.dma_start(out=wv_sb[:], in_=wv_d[:])
    nc.gpsimd.dma_start(out=bv_sb[:], in_=_bcast_ap(bv_d[:], P))

    qT_sb = [persist.tile([P, S], BF16, tag=f"qT{b}", name=f"qT{b}") for b in range(KB)]
    kT_sb = [persist.tile([P, S], BF16, tag=f"kT{b}", name=f"kT{b}") for b in range(KB)]
    v80_sb = [persist.tile([P, H, VW], BF16, tag=f"v80_{j}", name=f"v80_{j}") for j in range(SB)]

    # ---- pools (PSUM budget: pp 2 + slabs 4 + ao 2 = 8 banks) ----
    ptpool = ctx.enter_context(tc.tile_pool(name="ptpool", bufs=24))
    outTpool = ctx.enter_context(tc.tile_pool(name="outTpool", bufs=3))
    transpool = ctx.enter_context(tc.tile_pool(name="transpool", bufs=3))
    stagepool = ctx.enter_context(tc.tile_pool(name="stagepool", bufs=1))
    rpool = ctx.enter_context(tc.tile_pool(name="rpool", bufs=3))
    # one PSUM pool: tag "pp" (proj/attnV/warmup chains) gets 4 one-bank slots,
    # tag "slab" (energy pair slabs) gets 2 two-bank slots -> 8 banks total
    ppsum = ctx.enter_context(tc.tile_pool(name="ppsum", bufs=4, space="PSUM"))
    epsum = ppsum
    apsum = ppsum

    # normalized output staged in SBUF: [p, j, h, d]; written per-head (strided),
    # stored per row-block (contiguous) at the end -- keeps copy-DMAs away from
    # the xbar transposes (global DMATranspose<->DMACopy serialization)
    stage_sb = stagepool.tile([P, SB, H, HD], BF16, tag="stage", name="stage")

    def proj_qk(b):
        """projection of dout-block b for q and k (c0 of both first, so the
        first energy slab's inputs are ready earliest)"""
        for t, (w_sb, b_sb, dst) in enumerate(((wq_sb, bq_sb, qT_sb), (wk_sb, bk_sb, kT_sb))):
            for c in range(2):
                ps = ppsum.tile([P, 512], F32, tag="pp", name=f"pp{b}_{t}_{c}")
                for k in range(KB):
                    nc.tensor.matmul(
                        ps[:],
                        w_sb[:, k, b * P:(b + 1) * P],
                        xT_sb[:, k, c * 512:(c + 1) * 512],
                        start=(k == 0), stop=(k == KB - 1),
                    )
                nc.vector.tensor_scalar_add(
                    out=dst[b][:, c * 512:(c + 1) * 512],
                    in0=ps[:],
                    scalar1=b_sb[:, b:b + 1],
                )

    def proj_v():
        for j in range(SB):
            # zero pad cols + ones column (written once, before the data evac)
            nc.vector.memset(v80_sb[j][:, :, 64:VW], 0.0)
            nc.vector.memset(v80_sb[j][:, :, 64:65], 1.0)
            pv = ppsum.tile([P, 512], F32, tag="pp", name=f"pv{j}")
            for k in range(KB):
                nc.tensor.matmul(
                    pv[:],
                    xT_sb[:, k, j * P:(j + 1) * P],
                    wv_sb[:, k, :],
                    start=(k == 0), stop=(k == KB - 1),
                )
            nc.vector.tensor_add(
                out=v80_sb[j][:, :, 0:64],
                in0=pv[:].rearrange("p (h d) -> p h d", h=H),
                in1=bv_sb[:].rearrange("p (h d) -> p h d", h=H),
            )

    def energy_exp(hp, pt):
        """energy + exp for head pair hp; fills pt[j] tiles [P, 2, S]"""
        for j in range(SB):
            for c in range(2):
                # one slab holds both heads' chunk: rows 0-63 / 64-127 of the
                # PE array compute the two heads CONCURRENTLY (row tiling)
                slab = epsum.tile([P, 2, 512], F32, tag="slab", name=f"slab{hp}_{j}_{c}", bufs=2)
                for h01 in range(2):
                    rows = slice(h01 * 64, h01 * 64 + 64)
                    nc.tensor.matmul(
                        slab[:, h01, :],
                        kT_sb[hp][rows, j * P:(j + 1) * P],
                        qT_sb[hp][rows, c * 512:(c + 1) * 512],
                        start=True, stop=True,
                        tile_position=(h01 * 64, 0),
                    )
                nc.scalar.activation(
                    out=pt[j][:, c, :, :],
                    in_=slab[:],
                    func=mybir.ActivationFunctionType.Exp,
                    scale=SCALE,
                )

    def attn_v(hp, pt):
        for h01 in range(2):
            h = 2 * hp + h01
            outT = outTpool.tile([VW, S], BF16, tag="outT", name=f"outT{h}")
            for c in range(2):
                ao = apsum.tile([VW, 512], F32, tag="pp", name=f"ao{h}_{c}")
                for j in range(SB):
                    nc.tensor.matmul(
                        ao[:],
                        v80_sb[j][:, h, :],
                        pt[j][:, c, h01, :],
                        start=(j == 0), stop=(j == SB - 1),
                    )
                nc.vector.tensor_copy(out=outT[:, c * 512:(c + 1) * 512], in_=ao[:])
            # transpose back to [sq, VW] (one xbar op per head: ~1.2us fixed cost)
            trans = transpool.tile([P, SB, VW], BF16, tag="trans", name=f"trans{h}")
            nc.sync.dma_start_transpose(out=trans[:], in_=outT[:])
            rc = rpool.tile([P, SB, 1], F32, tag="rc", name=f"rc{h}")
            nc.vector.reciprocal(out=rc[:], in_=trans[:, :, 64:65])
            nc.vector.tensor_mul(
                out=stage_sb[:, :, h, :],
                in0=trans[:, :, 0:64],
                in1=rc[:].to_broadcast((P, SB, HD)),
            )

    # ---- emission order: get exp (ScalarE, the critical engine) started as
    # early as possible; PE fills waits with projections / attn@V ----
    def new_pts(hp):
        return [ptpool.tile([P, 2, 2, 512], BF16, tag="pt", name=f"pt{hp}_{j}") for j in range(SB)]

    # HAM warm-up: dummy matmuls on zeros while input DMAs run, so the PE
    # clock-gate is already released (2.4 GHz) when the real stream starts
    zt = consts.tile([P, 512], BF16, tag="zt", name="zt")
    nc.vector.memset(zt[:], 0.0)
    zp = ppsum.tile([P, 512], F32, tag="pp", name="warm")
    for w in range(16):
        nc.tensor.matmul(zp[:], zt[:, 0:P], zt[:], start=(w == 0), stop=(w == 15))

    # energy(hp+1) is emitted before attn_v(hp) so ScalarE (the critical
    # engine) never waits on lower-priority PE work at pair transitions
    proj_qk(0)
    pt0 = new_pts(0)
    energy_exp(0, pt0)
    proj_qk(1)
    pt1 = new_pts(1)
    energy_exp(1, pt1)
    proj_v()
    attn_v(0, pt0)
    proj_qk(2)
    pt2 = new_pts(2)
    energy_exp(2, pt2)
    attn_v(1, pt1)
    proj_qk(3)
    pt3 = new_pts(3)
    energy_exp(3, pt3)
    attn_v(2, pt2)
    attn_v(3, pt3)

    # final stores as 3 large DMAs (one per queue) -- minimal issue tail
    out_v3 = out_d[:].rearrange("(j p) e -> p j e", p=P)
    for j0, j1, eng in ((0, 3, nc.sync), (3, 6, nc.scalar), (6, 8, nc.gpsimd)):
        eng.dma_start(
            out=out_v3[:, j0:j1, :],
            in_=stage_sb[:, j0:j1, :, :].rearrange("p j h d -> p j (h d)"),
        )



_NC_CACHE = {}


def build_nc():
    if "nc" in _NC_CACHE:
        return _NC_CACHE["nc"]
    nc = bacc.Bacc("TRN2", target_bir_lowering=False, debug=False, num_devices=8)
    with tile.TileContext(nc) as tc:
        with ExitStack() as ctx:
            emit_kernel(ctx, tc)
    nc.compile()
    _NC_CACHE["nc"] = nc
    return nc


def host_prep(x, W_cq, b_cq, W_ck, b_ck, W_cv, b_cv, W_pq, b_pq, W_pk, b_pk, W_pv, b_pv):
    """Host-side sharding: exact f32 weight/bias fusion, bf16 casts, x transpose."""
    def blockw(a, b2):
        w = (np.asarray(a, np.float32) + np.asarray(b2, np.float32)).astype(NPBF16)
        return np.ascontiguousarray(w.reshape(KB, P, D).transpose(1, 0, 2))

    wq = blockw(W_cq, W_pq)
    wk = blockw(W_ck, W_pk)
    wv = blockw(W_cv, W_pv)
    bq = (np.asarray(b_cq, np.float32) + np.asarray(b_pq, np.float32)).reshape(KB, P).T.copy()
    bk = (np.asarray(b_ck, np.float32) + np.asarray(b_pk, np.float32)).reshape(KB, P).T.copy()
    bv = (np.asarray(b_cv, np.float32) + np.asarray(b_pv, np.float32)).reshape(1, D).copy()
    x = np.asarray(x, np.float32)
    in_maps = []
    for n in range(B):
        xT = np.ascontiguousarray(
            x[n].T.astype(NPBF16).reshape(KB, P, S).transpose(1, 0, 2))
        in_maps.append({
            "xT": xT, "wq": wq, "wk": wk, "wv": wv,
            "bq": bq, "bk": bk, "bv": bv,
        })
    return in_maps


def kernel(**inputs) -> np.ndarray:
    in_maps = host_prep(**inputs)
    nc = build_nc()
    res = run_bass_kernel_spmd(
        nc, in_maps, core_ids=list(range(B)),
        trace=bool(int(os.environ.get("KERNEL_TRACE", "0"))),
    )
    if res.exec_time_ns is not None:
        print(f"HW exec time: {res.exec_time_ns} ns")
    out = np.stack([res.results[i]["out"] for i in range(B)], axis=0)
    return out.astype(np.float32)



# revision 2
# speedup vs baseline: 1.0081x; 1.0081x over previous
"""Disentangled attention (fused common+personal QKV projections + MHA) on 8 TRN2 cores.

Strategy: data-parallel over batch N=8 (one batch element per NeuronCore, zero
communication). Host pre-sums W_c+W_p / b_c+b_p (exact), casts x/W to bf16, and
pre-transposes x so the device only sees x^T.

Per-core device pipeline (S=1024, D=512, H=8, hd=64):
  phase 1: qT = (W_q)^T-style projections producing q^T,k^T [D,S] and v [S,D]
           (bf16 matmuls, fp32 PSUM accumulate, bias added on PSUM evacuation)
  phase 2: per head: energy^T[sk,sq] = (kT tile)^T-free matmuls; exp on ScalarE
           (scale 1/sqrt(D) folded into activation; softmax max-subtraction
           skipped -- |energy/sqrt(D)| <~ 4.5 for these inputs); a subset of
           slabs is instead exp'd on VectorE via a Schraudolph bf16-bit trick
           (int16(e*A+B) reinterpreted as bf16) to relieve the ScalarE floor;
           attn@V with stationary [v_h | ones | 0-pad] (80 cols) giving
           out'^T[80,sq] with row 64 = softmax denominator; DMA-xbar transpose
           back to [sq,80]; VectorE reciprocal + broadcast multiply normalizes.

Load order is criticality-sorted (xT c0-half + wq/wk block0 first) so the first
projection chain starts ~5us earlier; outputs are stored per head-pair as they
complete instead of all at the end.
"""

import math
import os
from contextlib import ExitStack

import numpy as np
import ml_dtypes

import concourse.bass as bass
import concourse.tile as tile
import concourse.mybir as mybir
from concourse import bacc
from concourse.bass_utils import run_bass_kernel_spmd

B, S, D, H, HD = 8, 1024, 512, 8, 64
P = 128
KB = D // P           # 4 contraction blocks
SB = S // P           # 8 sequence tiles
VW = 80               # v-tile width: 64 data + 1 ones + 15 pad (xbar needs %16)
BF16 = mybir.dt.bfloat16
F32 = mybir.dt.float32
I16 = mybir.dt.int16
SCALE = 1.0 / float(np.sqrt(D))

# Schraudolph-style exp on the DVE: bf16 bit pattern of exp(e*SCALE) is
# approximately int16(e*SCHRA_A + SCHRA_B) (linear-mantissa 2^x construction,
# max rel err ~3%). Used only for the DVE_SLABS subset of energy slabs.
SCHRA_A = 128.0 * math.log2(math.e) * SCALE
SCHRA_B = 16256.0 - 128.0 * 0.0430

# (hp, j) energy slabs whose exp runs on VectorE instead of ScalarE.
# 6 of 32 (hp,j) pairs = 12 of 64 ACT-sized slabs (18.75% of exp work).
# hp=3-heavy so the last head pair's exp finishes earliest (shorter tail).
DVE_SLABS = {(1, 7), (2, 7), (3, 4), (3, 5), (3, 6), (3, 7)}

NPBF16 = ml_dtypes.bfloat16


def _bcast_ap(ap, parts):
    """Broadcast a [1, ...] AP across `parts` partitions (stride-0 partition dim)."""
    return bass.AP(tensor=ap.tensor, offset=ap.offset, ap=[[0, parts]] + list(ap.ap[1:]))


def emit_kernel(ctx: ExitStack, tc: tile.TileContext):
    nc = tc.nc

    xT_d = nc.dram_tensor("xT", [P, KB, S], BF16, kind="ExternalInput")
    # wq/wk host-blocked [p, b(dout block), k, 128] so per-block loads are
    # contiguous and the critical block b=0 can be fetched first
    wq_d = nc.dram_tensor("wq", [P, KB, KB, P], BF16, kind="ExternalInput")
    wk_d = nc.dram_tensor("wk", [P, KB, KB, P], BF16, kind="ExternalInput")
    wv_d = nc.dram_tensor("wv", [P, KB, D], BF16, kind="ExternalInput")
    bq_d = nc.dram_tensor("bq", [P, KB], F32, kind="ExternalInput")
    bk_d = nc.dram_tensor("bk", [P, KB], F32, kind="ExternalInput")
    bv_d = nc.dram_tensor("bv", [1, D], F32, kind="ExternalInput")
    out_d = nc.dram_tensor("out", [S, D], BF16, kind="ExternalOutput")

    consts = ctx.enter_context(tc.tile_pool(name="consts", bufs=1))
    persist = ctx.enter_context(tc.tile_pool(name="persist", bufs=1))

    xT_sb = persist.tile([P, KB, S], BF16, tag="xT", name="xT")
    wq_sb = persist.tile([P, KB, KB, P], BF16, tag="wq", name="wq")
    wk_sb = persist.tile([P, KB, KB, P], BF16, tag="wk", name="wk")
    wv_sb = persist.tile([P, KB, D], BF16, tag="wv", name="wv")
    bq_sb = consts.tile([P, KB], F32, tag="bq", name="bq")
    bk_sb = consts.tile([P, KB], F32, tag="bk", name="bk")
    bv_sb = consts.tile([P, D], F32, tag="bv", name="bv")

    # ---- loads, criticality-sorted ----
    # first ACT needs: xT c0-half (all k), wq block0, wk block0.  Those go
    # first on their queues; everything else streams behind them.
    nc.sync.dma_start(out=bq_sb[:], in_=bq_d[:])
    nc.sync.dma_start(out=bk_sb[:], in_=bk_d[:])
    nc.sync.dma_start(out=xT_sb[:, 0:2, 0:512], in_=xT_d[:, 0:2, 0:512])
    nc.scalar.dma_start(out=wq_sb[:, 0], in_=wq_d[:, 0])
    nc.gpsimd.dma_start(out=wk_sb[:, 0], in_=wk_d[:, 0])
    nc.scalar.dma_start(out=xT_sb[:, 2:3, 0:512], in_=xT_d[:, 2:3, 0:512])
    nc.gpsimd.dma_start(out=xT_sb[:, 3:4, 0:512], in_=xT_d[:, 3:4, 0:512])
    nc.sync.dma_start(out=xT_sb[:, 0:2, 512:1024], in_=xT_d[:, 0:2, 512:1024])
    nc.scalar.dma_start(out=xT_sb[:, 2:3, 512:1024], in_=xT_d[:, 2:3, 512:1024])
    nc.gpsimd.dma_start(out=xT_sb[:, 3:4, 512:1024], in_=xT_d[:, 3:4, 512:1024])
    nc.scalar.dma_start(out=wq_sb[:, 1:4], in_=wq_d[:, 1:4])
    nc.gpsimd.dma_start(out=wk_sb[:, 1:4], in_=wk_d[:, 1:4])
    nc.scalar.dma_start(out=wv_sb[:, 0:2], in_=wv_d[:, 0:2])
    nc.gpsimd.dma_start(out=wv_sb[:, 2:4], in_=wv_d[:, 2:4])
    nc.gpsimd.dma_start(out=bv_sb[:], in_=_bcast_ap(bv_d[:], P))

    qT_sb = [persist.tile([P, S], BF16, tag=f"qT{b}", name=f"qT{b}") for b in range(KB)]
    kT_sb = [persist.tile([P, S], BF16, tag=f"kT{b}", name=f"kT{b}") for b in range(KB)]
    # single v tile: [p(sk), j, h, VW]; stationary slice is [:, j, h, :]
    v80_sb = persist.tile([P, SB, H, VW], BF16, tag="v80", name="v80")

    ptpool = ctx.enter_context(tc.tile_pool(name="ptpool", bufs=24))
    outTpool = ctx.enter_context(tc.tile_pool(name="outTpool", bufs=3))
    transpool = ctx.enter_context(tc.tile_pool(name="transpool", bufs=3))
    stagepool = ctx.enter_context(tc.tile_pool(name="stagepool", bufs=1))
    rpool = ctx.enter_context(tc.tile_pool(name="rpool", bufs=3))
    # PSUM budget: tag "pp" 4 one-bank slots + tag "slab" 2 two-bank slots = 8
    ppsum = ctx.enter_context(tc.tile_pool(name="ppsum", bufs=4, space="PSUM"))

    stage_sb = stagepool.tile([P, SB, H, HD], BF16, tag="stage", name="stage")
    out_v3 = out_d[:].rearrange("(j p) e -> p j e", p=P)

    # v80 constant columns: one zero-fill of the pad region, then the ones col
    nc.vector.memset(v80_sb[:, :, :, 64:VW], 0.0)
    nc.vector.memset(v80_sb[:, :, :, 64:65], 1.0)

    # HAM warm-up: dummy matmuls on zeros while input DMAs run (trimmed to 8 --
    # the in-order PE queue must not delay the first real projection)
    zt = consts.tile([P, 512], BF16, tag="zt", name="zt")
    nc.vector.memset(zt[:], 0.0)
    zp = ppsum.tile([P, 512], F32, tag="pp", name="warm")
    for w in range(8):
        nc.tensor.matmul(zp[:], zt[:, 0:P], zt[:], start=(w == 0), stop=(w == 7))

    def proj_qk(b):
        """projections of dout-block b for q and k; c=0 chains first so the
        first energy slab's inputs are ready earliest"""
        for c in range(2):
            for t, (w_sb, b_sb, dst) in enumerate(((wq_sb, bq_sb, qT_sb), (wk_sb, bk_sb, kT_sb))):
                ps = ppsum.tile([P, 512], F32, tag="pp", name=f"pp{b}_{t}_{c}")
                for k in range(KB):
                    nc.tensor.matmul(
                        ps[:],
                        w_sb[:, b, k, :],
                        xT_sb[:, k, c * 512:(c + 1) * 512],
                        start=(k == 0), stop=(k == KB - 1),
                    )
                nc.vector.tensor_scalar_add(
                    out=dst[b][:, c * 512:(c + 1) * 512],
                    in0=ps[:],
                    scalar1=b_sb[:, b:b + 1],
                )

    def proj_v():
        for j in range(SB):
            pv = ppsum.tile([P, 512], F32, tag="pp", name=f"pv{j}")
            for k in range(KB):
                nc.tensor.matmul(
                    pv[:],
                    xT_sb[:, k, j * P:(j + 1) * P],
                    wv_sb[:, k, :],
                    start=(k == 0), stop=(k == KB - 1),
                )
            nc.vector.tensor_add(
                out=v80_sb[:, j, :, 0:64],
                in0=pv[:].rearrange("p (h d) -> p h d", h=H),
                in1=bv_sb[:].rearrange("p (h d) -> p h d", h=H),
            )

    def new_pts(hp):
        return [
            ptpool.tile([P, 2, 2, 512], I16 if (hp, j) in DVE_SLABS else BF16,
                        tag="pt", name=f"pt{hp}_{j}")
            for j in range(SB)
        ]

    def pt_mv(pts, hp, j, c, h01):
        ap = pts[j][:, c, h01, :]
        return ap.bitcast(BF16) if (hp, j) in DVE_SLABS else ap

    def energy_slab(hp, pt, j, c):
        """one [P,2,512] energy slab (both heads, row-tiled) + its exp"""
        slab = ppsum.tile([P, 2, 512], F32, tag="slab", name=f"slab{hp}_{j}_{c}", bufs=2)
        for h01 in range(2):
            rows = slice(h01 * 64, h01 * 64 + 64)
            nc.tensor.matmul(
                slab[:, h01, :],
                kT_sb[hp][rows, j * P:(j + 1) * P],
                qT_sb[hp][rows, c * 512:(c + 1) * 512],
                start=True, stop=True,
                tile_position=(h01 * 64, 0),
            )
        if (hp, j) in DVE_SLABS:
            nc.vector.tensor_scalar(
                out=pt[j][:, c, :, :],
                in0=slab[:],
                scalar1=SCHRA_A,
                scalar2=SCHRA_B,
                op0=mybir.AluOpType.mult,
                op1=mybir.AluOpType.add,
            )
        else:
            nc.scalar.activation(
                out=pt[j][:, c, :, :],
                in_=slab[:],
                func=mybir.ActivationFunctionType.Exp,
                scale=SCALE,
            )

    def energy_exp(hp, pt):
        for j in range(SB):
            for c in range(2):
                energy_slab(hp, pt, j, c)

    def finish_head(hp, h01, outT):
        """xbar transpose + normalize + (on h01==1) store the head pair"""
        h = 2 * hp + h01
        trans = transpool.tile([P, SB, VW], BF16, tag="trans", name=f"trans{h}")
        nc.sync.dma_start_transpose(out=trans[:], in_=outT[:])
        rc = rpool.tile([P, SB, 1], F32, tag="rc", name=f"rc{h}")
        nc.vector.reciprocal(out=rc[:], in_=trans[:, :, 64:65])
        nc.vector.tensor_mul(
            out=stage_sb[:, :, h, :],
            in0=trans[:, :, 0:64],
            in1=rc[:].to_broadcast((P, SB, HD)),
        )
        if h01 == 1:
            eng = (nc.gpsimd, nc.scalar, nc.gpsimd, nc.scalar)[hp]
            eng.dma_start(
                out=out_v3[:, :, hp * P:(hp + 1) * P],
                in_=stage_sb[:, :, 2 * hp:2 * hp + 2, :].rearrange("p j h d -> p j (h d)"),
            )

    def attn_v(hp, pt):
        for h01 in range(2):
            h = 2 * hp + h01
            outT = outTpool.tile([VW, S], BF16, tag="outT", name=f"outT{h}")
            # both c accumulators live so consecutive matmuls share a stationary
            ao = [ppsum.tile([VW, 512], F32, tag="pp", name=f"ao{h}_{c}") for c in range(2)]
            for j in range(SB):
                for c in range(2):
                    nc.tensor.matmul(
                        ao[c][:],
                        v80_sb[:, j, h, :],
                        pt_mv(pt, hp, j, c, h01),
                        start=(j == 0), stop=(j == SB - 1),
                    )
            for c in range(2):
                nc.vector.tensor_copy(out=outT[:, c * 512:(c + 1) * 512], in_=ao[c][:])
            finish_head(hp, h01, outT)

    def hp3_era(pt3):
        """energy+exp for hp=3 with its c=0 attn@V chains interleaved one j
        behind (so the PE never waits on the just-issued exp); c=1 chains and
        the finish run after -- a ~4us tail instead of ~10."""
        hp = 3
        outT3 = [outTpool.tile([VW, S], BF16, tag="outT", name=f"outT3_{h01}") for h01 in range(2)]
        ao0 = [ppsum.tile([VW, 512], F32, tag="pp", name=f"ao3c0_{h01}") for h01 in range(2)]

        def attn3_c0(j):
            for h01 in range(2):
                nc.tensor.matmul(
                    ao0[h01][:],
                    v80_sb[:, j, 6 + h01, :],
                    pt_mv(pt3, hp, j, 0, h01),
                    start=(j == 0), stop=(j == SB - 1),
                )

        for j in range(SB):
            for c in range(2):
                energy_slab(hp, pt3, j, c)
            if j >= 1:
                attn3_c0(j - 1)
        attn3_c0(SB - 1)
        for h01 in range(2):
            nc.vector.tensor_copy(out=outT3[h01][:, 0:512], in_=ao0[h01][:])
        for h01 in range(2):
            ao1 = ppsum.tile([VW, 512], F32, tag="pp", name=f"ao3c1_{h01}")
            for j in range(SB):
                nc.tensor.matmul(
                    ao1[:],
                    v80_sb[:, j, 6 + h01, :],
                    pt_mv(pt3, hp, j, 1, h01),
                    start=(j == 0), stop=(j == SB - 1),
                )
            nc.vector.tensor_copy(out=outT3[h01][:, 512:1024], in_=ao1[:])
            finish_head(hp, h01, outT3[h01])

    # ---- emission order: keep ScalarE (critical mid-phase engine) fed ----
    proj_qk(0)
    pt0 = new_pts(0)
    energy_exp(0, pt0)
    proj_qk(1)
    pt1 = new_pts(1)
    energy_exp(1, pt1)
    proj_v()
    attn_v(0, pt0)
    proj_qk(2)
    pt2 = new_pts(2)
    energy_exp(2, pt2)
    attn_v(1, pt1)
    proj_qk(3)
    pt3 = new_pts(3)
    attn_v(2, pt2)
    hp3_era(pt3)


_NC_CACHE = {}


def build_nc():
    if "nc" in _NC_CACHE:
        return _NC_CACHE["nc"]
    nc = bacc.Bacc("TRN2", target_bir_lowering=False, debug=False, num_devices=8)
    with tile.TileContext(nc) as tc:
        with ExitStack() as ctx:
            emit_kernel(ctx, tc)
    nc.compile()
    _NC_CACHE["nc"] = nc
    return nc


def host_prep(x, W_cq, b_cq, W_ck, b_ck, W_cv, b_cv, W_pq, b_pq, W_pk, b_pk, W_pv, b_pv):
    """Host-side sharding: exact f32 weight/bias fusion, bf16 casts, x transpose."""
    def blockw_qk(a, b2):
        w = (np.asarray(a, np.float32) + np.asarray(b2, np.float32)).astype(NPBF16)
        # [D, D] -> [p, b(dout blk), k, 128]: w[k*128+p, b*128+d]
        return np.ascontiguousarray(
            w.reshape(KB, P, KB, P).transpose(1, 2, 0, 3))

    def blockw_v(a, b2):
        w = (np.asarray(a, np.float32) + np.asarray(b2, np.float32)).astype(NPBF16)
        return np.ascontiguousarray(w.reshape(KB, P, D).transpose(1, 0, 2))

    wq = blockw_qk(W_cq, W_pq)
    wk = blockw_qk(W_ck, W_pk)
    wv = blockw_v(W_cv, W_pv)
    bq = (np.asarray(b_cq, np.float32) + np.asarray(b_pq, np.float32)).reshape(KB, P).T.copy()
    bk = (np.asarray(b_ck, np.float32) + np.asarray(b_pk, np.float32)).reshape(KB, P).T.copy()
    bv = (np.asarray(b_cv, np.float32) + np.asarray(b_pv, np.float32)).reshape(1, D).copy()
    x = np.asarray(x, np.float32)
    in_maps = []
    for n in range(B):
        xT = np.ascontiguousarray(
            x[n].T.astype(NPBF16).reshape(KB, P, S).transpose(1, 0, 2))
        in_maps.append({
            "xT": xT, "wq": wq, "wk": wk, "wv": wv,
            "bq": bq, "bk": bk, "bv": bv,
        })
    return in_maps


def kernel(**inputs) -> np.ndarray:
    in_maps = host_prep(**inputs)
    nc = build_nc()
    res = run_bass_kernel_spmd(
        nc, in_maps, core_ids=list(range(B)),
        trace=bool(int(os.environ.get("KERNEL_TRACE", "0"))),
    )
    if res.exec_time_ns is not None:
        print(f"HW exec time: {res.exec_time_ns} ns")
    out = np.stack([res.results[i]["out"] for i in range(B)], axis=0)
    return out.astype(np.float32)
